# revision 59
# baseline (speedup 1.0000x reference)
"""ColBERT MaxSim loss kernel for Trainium2 (8 NeuronCores, SPMD).

Strategy: shard documents across the 8 cores (32 docs each); queries
replicated. The host pre-casts to fp16 AND pre-transposes to partition-major
[128, KT, T] so all device loads are contiguous linear DMA -- measured 3.4x
faster than HWDGE DMA-transpose on real HW (~97 GB/s effective for the
transpose path). Projection matmuls accumulate fp16 into a [128, GROUP, 64]
PSUM group tile; per-token L2 norms run on Act (Square+accum per block, one
batched Sqrt per group) with a batched DVE reciprocal; Act applies the
normalize scale (fp16 out), the PE transposes to [64, tokens], and Act
deposits two blocks per copy. The DVE is left almost exclusively to the
MaxSim reduce: fp16 sim matmuls (D=64) into fp32 PSUM, reduce-max over Ld
straight out of PSUM (measured 856ns per 4-doc piece, exactly the DVE
roofline). Two scheduling moves matter most: (1) sim pieces are emitted
BETWEEN a group's projections and its scale/transpose chain -- their data
deps are a group old, so the in-order PE queue always has stall-free work
while Act catches up (this broke a 140us head-of-line-blocking plateau);
(2) the per-group batched Sqrt+reciprocal get a scheduler priority boost so
the reciprocal jumps the DVE's in-order reduce backlog instead of adding
~1-3us of queueing latency to every group's norm chain (121.2 -> 110.9us).
An ablation with the sim work removed entirely runs SLOWER (162.7us) than
the full kernel: the norm chain is the latency-bound critical path and the
sim work is what fills its stalls. The Lq-sum uses a block-diagonal ones
matmul; the host concatenates the 8 [32 x 32] score blocks and finishes the
tiny cross-entropy in float64.

Measured (repeat-slope, 8 cores): 110.9us/iter (vs 139.7us baseline);
loss rel err 2.5e-5 (gate 2e-2).
"""

import sys

import numpy as np

try:
    import concourse.bass as bass
except ImportError:  # pragma: no cover
    sys.path.insert(0, "/opt/trn_rl_repo")
    import concourse.bass as bass

import concourse.mybir as mybir
import concourse.tile as tile
from concourse.bass_utils import run_bass_kernel_spmd
from concourse.masks import make_identity

F32 = mybir.dt.float32
F16 = mybir.dt.float16
F8 = mybir.dt.float8e4
W8_SCALE = 16.0  # pre-scale on W for the fp8 doc path; cancels in L2 norm

# Problem shape (hardcoded).
BQ, LQ, BD, LD, H, D = 32, 32, 256, 180, 768, 64
NCORES = 8
BD_LOC = BD // NCORES  # 32 docs per core
TD = BD_LOC * LD  # 5760 doc tokens per core
TQ = BQ * LQ  # 1024 query tokens
KT = H // 128  # 6 contraction k-tiles
NB_D = TD // 128  # 45 doc token blocks
NB_Q = TQ // 128  # 8 query token blocks
Q_PER_BLOCK = 128 // LQ  # 4 queries per 128-token block
SIM_CHUNK = 4  # docs per sim chunk (2 pair-matmuls of N=360)
N_CHUNKS = BD_LOC // SIM_CHUNK  # 8
DOC_PIECES = (720, 720, 1440, 1440, 1440)  # doc DMA-transpose piece schedule
Q_PIECE = 1024  # tokens per qry DMA-transpose piece
GROUP = 4  # token blocks per norm batch group
DEP_PACK = 2  # token blocks per deposit copy

PS_S_BUFS = 2
PD_BUFS = 2
TR_BUFS = 2
DN_BUFS = 4
SMALL_BUFS = 8

# Engine knobs (tunable): which engine runs the scale and the deposits.
SCALE_ENG = "act"
DEPOSIT_ENG = "act"
Q_ENG = "act"
D_ENG = "sp"
LINEAR_DMA = True  # host pre-transposes; device loads are contiguous
DOC_FP8 = False  # docs (and a pre-scaled W copy) in fp8e4; DoubleRow projection
SIM_SLACK = 0  # extra deposited tokens required before a chunk's sim emits
EMIT_SIM = True  # diagnostics: disable sim+reduce emission entirely
PRIO_BOOST = 100000  # scheduler priority offset for the latency-critical
# norm chain (squares/sqrt/recip/scales/transposes/deposits); 0 disables


def _engine(nc, name):
    return nc.scalar if name == "act" else (nc.vector if name == "dve" else nc.sync)


def _prio(tc):
    """Priority-boost context for latency-critical norm-chain ops."""
    from contextlib import nullcontext

    return tc.high_priority(offset=PRIO_BOOST) if PRIO_BOOST else nullcontext()


def _kernel_body(tc, dsl, qsl, wt, qmask, scores_out, wt8=None, repeat=1):
    nc = tc.nc
    with (
        tc.tile_pool(name="const", bufs=1) as const,
        tc.tile_pool(name="dn", bufs=DN_BUFS) as dn,
        tc.tile_pool(name="small", bufs=SMALL_BUFS) as small,
        tc.tile_pool(name="ps_pd", bufs=PD_BUFS, space="PSUM") as ps_pd,
        tc.tile_pool(name="ps_tr", bufs=TR_BUFS, space="PSUM") as ps_tr,
        tc.tile_pool(name="ps_s", bufs=PS_S_BUFS, space="PSUM") as ps_s,
    ):
        ident_f = const.tile([128, 128], F32)
        make_identity(nc, ident_f)
        ident16 = const.tile([128, 128], F16, name="identity16")
        nc.vector.tensor_copy(out=ident16, in_=ident_f)

        # W.T as 6 k-tiles: wt_sb[p, k, d] = W.T[k*128+p, d]
        wt_sb = const.tile([128, KT, D], F16)
        nc.sync.dma_start(
            out=wt_sb, in_=wt[:, :].rearrange("(k p) d -> p k d", p=128)
        )
        if DOC_FP8:
            wt8_sb = const.tile([128, KT, D], F8)
            nc.sync.dma_start(
                out=wt8_sb, in_=wt8[:, :].rearrange("(k p) d -> p k d", p=128)
            )
        qmask_sb = const.tile([128, NB_Q, BQ], F16)
        nc.sync.dma_start(out=qmask_sb, in_=qmask[:, :, :])

        qtk = const.tile([128, KT, TQ], F16)  # transposed raw queries
        dtk = const.tile([128, KT, TD], F8 if DOC_FP8 else F16)  # raw docs
        qt = const.tile([64, TQ], F16)  # normalized projected queries
        dt_ = const.tile([64, TD], F16)  # normalized projected docs
        maxsim_all = const.tile([128, NB_Q, BD_LOC], F16)

        # Token-block groups: first query half, first doc group, second query
        # half, then the remaining doc groups — so the first sim pieces (which
        # need q blocks + 6 doc blocks deposited) unlock as early as possible.
        qg = [
            [("q", b) for b in range(i, min(i + GROUP, NB_Q))]
            for i in range(0, NB_Q, GROUP)
        ]
        dg = [
            [("d", b) for b in range(i, min(i + GROUP, NB_D))]
            for i in range(0, NB_D, GROUP)
        ]
        groups = [qg[0], dg[0], dg[1]] + qg[1:] + dg[2:]

        def _one_pass():
            # Loads: first doc piece on SP concurrently with queries on Act,
            # then the remaining doc pieces on SP. With LINEAR_DMA the host
            # pre-transposed the slabs, so each piece is one contiguous
            # [128, KT*n] strip per k-tile; otherwise HWDGE DMA-transpose.
            deng = _engine(nc, D_ENG)
            qeng = _engine(nc, Q_ENG)

            def _doc_piece(t0, n):
                for k in range(KT):
                    if LINEAR_DMA:
                        deng.dma_start(
                            out=dtk[:, k, t0 : t0 + n],
                            in_=dsl[:, k, t0 : t0 + n],
                        )
                    else:
                        deng.dma_start(
                            out=dtk[:, k, t0 : t0 + n],
                            in_=dsl[k, t0 : t0 + n, :],
                            transpose=True,
                        )

            def _q_piece(t0, n):
                for k in range(KT):
                    if LINEAR_DMA:
                        qeng.dma_start(
                            out=qtk[:, k, t0 : t0 + n],
                            in_=qsl[:, k, t0 : t0 + n],
                        )
                    else:
                        qeng.dma_start(
                            out=qtk[:, k, t0 : t0 + n],
                            in_=qsl[k, t0 : t0 + n, :],
                            transpose=True,
                        )

            _doc_piece(0, DOC_PIECES[0])
            for p in range(TQ // Q_PIECE):
                _q_piece(p * Q_PIECE, Q_PIECE)
            t0 = DOC_PIECES[0]
            for n in DOC_PIECES[1:]:
                _doc_piece(t0, n)
                t0 += n

            seng = _engine(nc, SCALE_ENG)
            peng = _engine(nc, DEPOSIT_ENG)

            # Sim pacing state: emit pieces interleaved with doc groups.
            pending = []
            emitted = 0
            next_chunk = 0
            total_pieces = N_CHUNKS * NB_Q

            def _emit_sim_piece(c, qb, half=None):
                """MaxSim for docs [c*4, (c+1)*4) x query block qb.

                half=0/1 emits just one 2-doc matmul+reduce (finer pieces to
                prime the pipeline before the first full chunks are ready).
                """
                col0 = c * SIM_CHUNK * LD
                ps = ps_s.tile([128, 2, 512], F32, tag="sim")
                js = range(2) if half is None else [half]
                for j in js:
                    nc.tensor.matmul(
                        ps[:, j, 0:360],
                        lhsT=qt[:, qb * 128 : (qb + 1) * 128],
                        rhs=dt_[:, col0 + j * 360 : col0 + (j + 1) * 360],
                        start=True,
                        stop=True,
                    )
                for j in js:
                    out_view = maxsim_all[
                        :, qb, c * SIM_CHUNK + 2 * j : c * SIM_CHUNK + 2 * j + 2
                    ]
                    in_view = ps[:, j, 0:360].rearrange("p (d l) -> p d l", d=2)
                    nc.vector.reduce_max(
                        out=out_view, in_=in_view, axis=mybir.AxisListType.X
                    )

            done_doc_blocks = 0
            done_q_blocks = 0
            queued = set()  # (c, qb, half) already queued

            if not EMIT_SIM:
                # Diagnostics mode: keep the scores path alive.
                nc.vector.memset(maxsim_all, 0.0)

            def _pace_sim(last=False):
                nonlocal emitted
                if not EMIT_SIM:
                    return
                done_tokens = TD if last else done_doc_blocks * 128
                qb_ready = NB_Q if last else min(done_q_blocks, NB_Q)
                for c in range(N_CHUNKS):
                    base = c * SIM_CHUNK * LD + (0 if last else SIM_SLACK)
                    if c == 0 and base + 720 > done_tokens:
                        # Prime the pipeline with 2-doc halves of chunk 0.
                        for half in range(2):
                            if base + (half + 1) * 2 * LD > done_tokens:
                                break
                            for qb in range(qb_ready):
                                if (c, qb, half) not in queued:
                                    queued.add((c, qb, half))
                                    pending.append((c, qb, half))
                        continue
                    if base + SIM_CHUNK * LD > done_tokens:
                        break
                    for qb in range(qb_ready):
                        if (c, qb, 0) in queued:
                            if (c, qb, 1) not in queued:
                                queued.add((c, qb, 1))
                                pending.append((c, qb, 1))
                        elif (c, qb, None) not in queued:
                            queued.add((c, qb, None))
                            pending.append((c, qb, None))
                blocks_left = NB_D - done_doc_blocks
                total_left = total_pieces - emitted
                quota = (
                    len(pending)
                    if last
                    else max(0, -(-total_left // (blocks_left + 1)))
                )
                for _ in range(min(quota, len(pending))):
                    c, qb, half = pending.pop(0)
                    _emit_sim_piece(c, qb, half=half)
                    emitted += 1



            for grp in groups:
                g = len(grp)
                pd = ps_pd.tile([128, GROUP, D], F32, tag="pd")
                ssq = small.tile([128, GROUP], F32, tag="ssq")
                # Projection + per-block sum-of-squares.
                for i, (which, b) in enumerate(grp):
                    slab = qtk if which == "q" else dtk
                    if which == "d" and DOC_FP8:
                        # fp8 DoubleRow: two k-tiles per matmul.
                        for k in range(KT // 2):
                            nc.tensor.matmul(
                                pd[:, i, :],
                                lhsT=slab[:, 2 * k : 2 * k + 2, b * 128 : (b + 1) * 128],
                                rhs=wt8_sb[:, 2 * k : 2 * k + 2, :],
                                start=(k == 0),
                                stop=(k == KT // 2 - 1),
                                perf_mode=mybir.MatmulPerfMode.DoubleRow,
                            )
                    else:
                        for k in range(KT):
                            nc.tensor.matmul(
                                pd[:, i, :],
                                lhsT=slab[:, k, b * 128 : (b + 1) * 128],
                                rhs=wt_sb[:, k, :],
                                start=(k == 0),
                                stop=(k == KT - 1),
                            )
                    sq_scratch = dn.tile([128, D], F16, tag="sqs")
                    nc.scalar.activation(
                        out=sq_scratch,
                        in_=pd[:, i, :],
                        func=mybir.ActivationFunctionType.Square,
                        accum_out=ssq[:, i : i + 1],
                    )
                # Fill the PE queue with sim pieces (old deps, no stalls)
                # while Act works through this group's squares/scales.
                _pace_sim()
                # Batched 1/sqrt for the whole group.
                nrm = small.tile([128, GROUP], F32, tag="nrm")
                rn = small.tile([128, GROUP], F32, tag="rn")
                with _prio(tc):
                    nc.scalar.activation(
                        out=nrm[:, 0:g],
                        in_=ssq[:, 0:g],
                        func=mybir.ActivationFunctionType.Sqrt,
                    )
                    nc.vector.reciprocal(out=rn[:, 0:g], in_=nrm[:, 0:g])
                # Scale (fp16), transpose, deposit in pairs of blocks.
                is_last = grp is groups[-1]
                i = 0
                while i < g:
                    npair = min(DEP_PACK, g - i)
                    ptr = ps_tr.tile([64, DEP_PACK, 128], F16, tag="ptr")
                    for j in range(npair):
                        which, b = grp[i + j]
                        dnrm = dn.tile([128, D], F16, tag="dnrm")
                        nc.scalar.activation(
                            out=dnrm,
                            in_=pd[:, i + j, :],
                            func=mybir.ActivationFunctionType.Copy,
                            scale=rn[:, i + j : i + j + 1],
                        )
                        nc.tensor.transpose(ptr[:, j, :], dnrm, ident16)
                    which, b = grp[i]
                    out_t = qt if which == "q" else dt_
                    deng_ = nc.scalar if DEPOSIT_ENG == "act" else nc.vector
                    if DEPOSIT_ENG == "act":
                        deng_.copy(
                            out=out_t[:, b * 128 : (b + npair) * 128],
                            in_=ptr[:, 0:npair, :],
                        )
                    else:
                        deng_.tensor_copy(
                            out=out_t[:, b * 128 : (b + npair) * 128],
                            in_=ptr[:, 0:npair, :],
                        )
                    for j in range(npair):
                        if grp[i + j][0] == "d":
                            done_doc_blocks += 1
                        else:
                            done_q_blocks += 1
                    _pace_sim(last=(is_last and i + npair >= g))
                    i += npair

            # Lq-sum via block-diag ones: scores[q, d] = sum_i maxsim[q*32+i, d]
            scores_sb = small.tile([BQ, BD_LOC], F32, tag="scores")
            scores_ps = ps_s.tile([BQ, BD_LOC], F32, tag="sim")
            for qb in range(NB_Q):
                nc.tensor.matmul(
                    scores_ps,
                    lhsT=qmask_sb[:, qb, :],
                    rhs=maxsim_all[:, qb, :],
                    start=(qb == 0),
                    stop=(qb == NB_Q - 1),
                )
            nc.vector.tensor_copy(out=scores_sb, in_=scores_ps)
            nc.sync.dma_start(out=scores_out[:, :], in_=scores_sb)

        if repeat == 1:
            _one_pass()
        else:
            with tc.For_i(0, repeat, 1):
                _one_pass()


def split_multi_waits(nc, max_waits=1):
    """Public neuronxcc walrus encodes one inline sync-wait per instruction;
    split excess waits into preceding same-engine nop-waits."""
    for f in nc.m.functions:
        for blk in f.blocks:
            new_insts = []
            for inst in blk.instructions:
                si = inst.sync_info
                if si is not None and len(si.on_wait) > max_waits:
                    waits = list(si.on_wait)
                    for w in waits[:-max_waits]:
                        new_insts.append(
                            mybir.InstNoOp(
                                name=nc.get_next_instruction_name(),
                                ins=[],
                                outs=[],
                                engine=inst.engine,
                                sync_info=mybir.SyncInfo(on_wait=[w], on_update=[]),
                            )
                        )
                    inst.sync_info = mybir.SyncInfo(
                        on_wait=waits[-max_waits:], on_update=list(si.on_update)
                    )
                new_insts.append(inst)
            blk.instructions = new_insts
    return nc


def build_bass(repeat=1, split_waits=True, **knobs):
    global SCALE_ENG, DEPOSIT_ENG, Q_ENG, D_ENG, DOC_PIECES, GROUP
    global PS_S_BUFS, PD_BUFS, TR_BUFS, DN_BUFS, SMALL_BUFS, LINEAR_DMA, DOC_FP8
    names = [
        "SCALE_ENG", "DEPOSIT_ENG", "Q_ENG", "D_ENG", "DOC_PIECES", "GROUP",
        "PS_S_BUFS", "PD_BUFS", "TR_BUFS", "DN_BUFS", "SMALL_BUFS", "LINEAR_DMA",
        "DOC_FP8", "SIM_SLACK", "EMIT_SIM", "DEP_PACK", "PRIO_BOOST",
    ]
    g = globals()
    old = {n: g[n] for n in names}
    for k, v in knobs.items():
        ku = k.upper()
        assert ku in names, f"unknown knob {k}"
        g[ku] = v
    try:
        nc = bass.Bass()
        ddt = F8 if DOC_FP8 else F16
        if LINEAR_DMA:
            dsl = nc.dram_tensor("dsl", [128, KT, TD], ddt, kind="ExternalInput")
            qsl = nc.dram_tensor("qsl", [128, KT, TQ], F16, kind="ExternalInput")
        else:
            assert not DOC_FP8, "fp8 docs require the linear-DMA layout"
            dsl = nc.dram_tensor("dsl", [KT, TD, 128], F16, kind="ExternalInput")
            qsl = nc.dram_tensor("qsl", [KT, TQ, 128], F16, kind="ExternalInput")
        wt = nc.dram_tensor("wt", [H, D], F16, kind="ExternalInput")
        wt8 = (
            nc.dram_tensor("wt8", [H, D], F8, kind="ExternalInput")
            if DOC_FP8
            else None
        )
        qmask = nc.dram_tensor("qmask", [128, NB_Q, BQ], F16, kind="ExternalInput")
        scores_out = nc.dram_tensor(
            "scores", [BQ, BD_LOC], F32, kind="ExternalOutput"
        )
        with tile.TileContext(nc) as tc:
            _kernel_body(tc, dsl, qsl, wt, qmask, scores_out, wt8=wt8, repeat=repeat)
        if split_waits:
            split_multi_waits(nc)
        return nc
    finally:
        for n in names:
            g[n] = old[n]


def _build_qmask():
    qmask = np.zeros((128, NB_Q, BQ), dtype=np.float16)
    p = np.arange(128)
    for qb in range(NB_Q):
        qmask[p, qb, qb * Q_PER_BLOCK + p // LQ] = 1.0
    return qmask


_NC_CACHE = None


def _get_nc():
    global _NC_CACHE
    if _NC_CACHE is None:
        _NC_CACHE = build_bass()
    return _NC_CACHE


def _ktile_major(x2d):
    """[T, H] fp32 -> [KT, T, 128] fp16 contiguous (DMA-transpose layout)."""
    T = x2d.shape[0]
    return np.ascontiguousarray(
        x2d.reshape(T, KT, 128).transpose(1, 0, 2).astype(np.float16)
    )


def _part_major(x2d):
    """[T, H] fp32 -> [128, KT, T] fp16 contiguous (linear-DMA layout).

    Partition p holds H-components {k*128+p} for all tokens, so each
    [128, n] strip loads as one contiguous run per partition.
    """
    T = x2d.shape[0]
    return np.ascontiguousarray(
        x2d.reshape(T, KT, 128).astype(np.float16).transpose(2, 1, 0)
    )


def _part_major8(x2d):
    """[T, H] fp32 -> [128, KT, T] fp8e4 contiguous."""
    T = x2d.shape[0]
    f8 = mybir.dt.np(F8)
    return np.ascontiguousarray(
        x2d.reshape(T, KT, 128).astype(f8).transpose(2, 1, 0)
    )


def _make_in_maps(qry_emb, doc_emb, W):
    lay = _part_major if LINEAR_DMA else _ktile_major
    dlay = _part_major8 if DOC_FP8 else lay
    wt = np.ascontiguousarray(W.T.astype(np.float16))  # [768, 64]
    qsl = lay(qry_emb.reshape(TQ, H))
    qmask = _build_qmask()
    shared = {"qsl": qsl, "wt": wt, "qmask": qmask}
    if DOC_FP8:
        shared["wt8"] = np.ascontiguousarray(
            (W.T * W8_SCALE).astype(mybir.dt.np(F8))
        )
    in_maps = []
    for c in range(NCORES):
        dsl = dlay(doc_emb[c * BD_LOC : (c + 1) * BD_LOC].reshape(TD, H))
        in_maps.append({"dsl": dsl, **shared})
    return in_maps


def _finish_loss(score_blocks, group_size):
    scores = np.concatenate(score_blocks, axis=1).astype(np.float64)  # [32, 256]
    labels = np.arange(BQ) * int(group_size)
    m = scores.max(axis=1, keepdims=True)
    lse = m[:, 0] + np.log(np.exp(scores - m).sum(axis=1))
    loss = np.mean(lse - scores[np.arange(BQ), labels])
    return np.float32(loss)


def kernel(qry_emb, doc_emb, W, group_size, _trace=False):
    nc = _get_nc()
    in_maps = _make_in_maps(np.asarray(qry_emb), np.asarray(doc_emb), np.asarray(W))
    res = run_bass_kernel_spmd(nc, in_maps, list(range(NCORES)), trace=_trace)
    blocks = [res.results[c]["scores"] for c in range(NCORES)]
    loss = _finish_loss(blocks, group_size)
    if _trace:
        return loss, res
    return loss
